# revision 19
# baseline (speedup 1.0000x reference)
"""BiLSTM tagger kernel for 8 Trainium2 NeuronCores.

Strategy: data-parallel over batch (16 sequences per core, weights
replicated). Per core, the two directions of each BiLSTM layer run as
interleaved scans so gate math on ScalarE/VectorE hides under the other
scan's recurrent matmul on TensorE. All matmuls run in bf16 (fp32 matmul
is 4x slower on TRN2); PSUM accumulation stays fp32.

Recurrent step layout: stationary = h^T chunks [128,16], moving = W_hh^T
slices, psum gates [16, 2048]. gx (input projections, precomputed per
layer into HBM) is added on VectorE during the psum drain. h is
re-transposed each step with four tiny matmuls against a 16x16 identity.
Backward scans consume inputs pre-reversed per sequence length (host
permutation indices + indirect DMA); their outputs are scattered back
through the same permutation, which also writes the zero padding the
reference produces. The permutation is t -> len-1-t for t < len, else
t -> t; steps past len compute garbage that is masked to zero and cannot
contaminate earlier steps.
"""

import sys

for _p in ("/opt/trn_rl_repo",):
    if _p not in sys.path:
        sys.path.append(_p)

import numpy as np
import ml_dtypes

import concourse.bass as bass
import concourse.tile as tile
from concourse import bacc, mybir
from concourse.bass import IndirectOffsetOnAxis
from concourse.bass_utils import run_bass_kernel_spmd

F32 = mybir.dt.float32
BF16 = mybir.dt.bfloat16
I32 = mybir.dt.int32
AF = mybir.ActivationFunctionType
ALU = mybir.AluOpType

# problem sizes (full / per-core)
B, T, V, E, H, TAGS = 128, 512, 50000, 256, 512, 64
NC = 8
BL = B // NC   # 16 sequences per core
G = 4 * H      # 2048 gate width

# permutation taking pytorch gate order i,f,g,o -> i,f,o,g (sigmoid block first)
_GATE_PERM = np.concatenate([
    np.arange(0, H), np.arange(H, 2 * H), np.arange(3 * H, 4 * H),
    np.arange(2 * H, 3 * H)])


def _build(nc, Tn=T, Bl=BL, TC=2, RC=4):
    """Emit the per-core program. Tn shrinkable for dev testing."""
    ntok = Bl * Tn
    nchunk = ntok // 128
    KE = E // 128       # k-chunks for layer-1 input proj
    KH2 = 2 * H // 128  # k-chunks for layer-2 input proj / classifier
    KH = H // 128       # k-chunks for recurrent
    assert ntok % 128 == 0

    # ---- dram I/O ----
    emb = nc.dram_tensor("emb", [V, E], F32, kind="ExternalInput")
    xf_idx = nc.dram_tensor("xf_idx", [128, nchunk], I32, kind="ExternalInput")
    xb_idx = nc.dram_tensor("xb_idx", [128, nchunk], I32, kind="ExternalInput")
    rev128 = nc.dram_tensor("rev128", [128, nchunk], I32, kind="ExternalInput")
    rev16 = nc.dram_tensor("rev16", [Bl, Tn], I32, kind="ExternalInput")
    mask = nc.dram_tensor("mask", [Bl, Tn], F32, kind="ExternalInput")
    ident = nc.dram_tensor("ident", [16, 16], BF16, kind="ExternalInput")

    wih, whh, biasd = {}, {}, {}
    for s, din in (("f1", E), ("b1", E), ("f2", 2 * H), ("b2", 2 * H)):
        wih[s] = nc.dram_tensor(f"wihT_{s}", [din, G], BF16, kind="ExternalInput")
        whh[s] = nc.dram_tensor(f"whhT_{s}", [H, G], BF16, kind="ExternalInput")
        biasd[s] = nc.dram_tensor(f"bias_{s}", [128, G], F32, kind="ExternalInput")
    wcls = nc.dram_tensor("wclsT", [2 * H, TAGS], BF16, kind="ExternalInput")
    bcls = nc.dram_tensor("bcls", [TAGS, 1], F32, kind="ExternalInput")

    gx = {s: nc.dram_tensor(f"gx_{s}", [ntok, G], BF16)
          for s in ("f1", "b1", "f2", "b2")}
    # per-direction layer outputs; backward halves stay in scan order and are
    # un-reversed by the consumers' row gathers (no per-step scatters)
    hout = {s: nc.dram_tensor(f"hout_{s}", [ntok, H], BF16)
            for s in ("f1", "b1", "f2", "b2")}
    logitsT = nc.dram_tensor("logitsT", [TAGS, ntok], F32, kind="ExternalOutput")

    with tile.TileContext(nc) as tc:
        with tc.tile_pool(name="const", bufs=1) as cpool:
            def load_const(nm, shape, dt, src_ap):
                t = cpool.tile(shape, dt, name=nm, tag=nm)
                nc.gpsimd.dma_start(t[:], src_ap)
                return t

            xf_sb = load_const("xf_sb", [128, nchunk], I32, xf_idx[:])
            xb_sb = load_const("xb_sb", [128, nchunk], I32, xb_idx[:])
            rev128_sb = load_const("rev128_sb", [128, nchunk], I32, rev128[:])
            rev16_sb = load_const("rev16_sb", [Bl, Tn], I32, rev16[:])
            mask_sb = load_const("mask_sb", [Bl, Tn], F32, mask[:])
            id_sb = load_const("id_sb", [16, 16], BF16, ident[:])
            bcls_sb = load_const("bcls_sb", [TAGS, 1], F32, bcls[:])
            bias_sb = {s: load_const(f"bias_sb_{s}", [128, G], F32, biasd[s][:])
                       for s in ("f1", "b1", "f2", "b2")}
            wcls_sb = cpool.tile([128, KH2, TAGS], BF16, name="wcls_sb")
            for k in range(KH2):
                nc.gpsimd.dma_start(wcls_sb[:, k, :], wcls[128 * k:128 * (k + 1), :])

            # layer-1 input projections (inputs gathered from embedding table)
            _proj_phase(nc, tc, nchunk, KE, wih=wih, bias_sb=bias_sb, gx=gx,
                        jobs=[("f1", emb, xf_sb, True), ("b1", emb, xb_sb, True)])
            # layer-1 scans
            _scan_phase(nc, tc, Tn, Bl, TC, RC, KH,
                        scans=("f1", "b1"), whh=whh, gx=gx, hout=hout,
                        mask_sb=mask_sb, id_sb=id_sb)
            # layer-2 input projections: input token (b,t) for the fwd scan is
            # [f1h[t], s1h[rev(t)]]; for the bwd scan it is [f1h[rev(t)], s1h[t]]
            _proj_phase(nc, tc, nchunk, KH2, wih=wih, bias_sb=bias_sb, gx=gx,
                        jobs=[("f2", (hout["f1"], None, hout["b1"], rev128_sb), None, False),
                              ("b2", (hout["f1"], rev128_sb, hout["b1"], None), None, False)])
            # layer-2 scans
            _scan_phase(nc, tc, Tn, Bl, TC, RC, KH,
                        scans=("f2", "b2"), whh=whh, gx=gx, hout=hout,
                        mask_sb=mask_sb, id_sb=id_sb)

            # classifier: logits^T = W_cls @ out2^T + b_cls
            with tc.tile_pool(name="cls", bufs=3) as gp, \
                 tc.tile_pool(name="clsT", bufs=3) as gtp, \
                 tc.tile_pool(name="clsps", bufs=4, space="PSUM") as pp, \
                 tc.tile_pool(name="clso", bufs=3) as op:
                for c in range(nchunk):
                    o2 = gp.tile([128, 2 * H], BF16, tag="in")
                    nc.gpsimd.dma_start(o2[:, 0:H], hout["f2"][128 * c:128 * (c + 1), :])
                    nc.gpsimd.indirect_dma_start(
                        out=o2[:, H:2 * H], out_offset=None, in_=hout["b2"][:],
                        in_offset=IndirectOffsetOnAxis(ap=rev128_sb[:, c:c + 1], axis=0))
                    o2T = gtp.tile([128, KH2, 128], BF16, tag="inT")
                    for k in range(KH2):
                        nc.sync.dma_start_transpose(
                            o2T[:, k, :], o2[:, 128 * k:128 * (k + 1)])
                    ps = pp.tile([TAGS, 128], F32, name="clsps_t")
                    for k in range(KH2):
                        nc.tensor.matmul(ps[:], wcls_sb[:, k, :], o2T[:, k, :],
                                         start=(k == 0), stop=(k == KH2 - 1))
                    lg = op.tile([TAGS, 128], F32, tag="lg")
                    nc.scalar.activation(lg[:], ps[:], AF.Identity,
                                         bias=bcls_sb[:, 0:1])
                    nc.gpsimd.dma_start(logitsT[:, 128 * c:128 * (c + 1)], lg[:])
    return nc


def _proj_phase(nc, tc, nchunk, KD, wih, bias_sb, gx, jobs):
    """gx_s = input @ W_ih_s^T + b_s, written contiguously in scan-time order.

    jobs: (scan_name, dram_src, idx_tile_or_None, is_emb). For is_emb the idx
    tile holds embedding row ids (fp32 gather + cast); otherwise rows of src
    are read contiguously (idx None) or gathered (idx set, layer-2 backward).
    """
    D = KD * 128
    with tc.tile_pool(name="pw", bufs=1) as wpool, \
         tc.tile_pool(name="pg", bufs=3) as gpool, \
         tc.tile_pool(name="pgT", bufs=3) as tpool, \
         tc.tile_pool(name="pps", bufs=4, space="PSUM") as ppool, \
         tc.tile_pool(name="pout", bufs=3) as opool:
        wsb = {}
        for s, _, _, _ in jobs:
            wsb[s] = wpool.tile([128, KD, G], BF16, tag=f"w{s}", name=f"wih_{s}")
            for k in range(KD):
                nc.gpsimd.dma_start(wsb[s][:, k, :], wih[s][128 * k:128 * (k + 1), :])
        for c in range(nchunk):
            for s, dsrc, idx, is_emb in jobs:
                if is_emb:
                    e32 = gpool.tile([128, D], F32, tag="e32")
                    nc.gpsimd.indirect_dma_start(
                        out=e32[:], out_offset=None, in_=dsrc[:],
                        in_offset=IndirectOffsetOnAxis(ap=idx[:, c:c + 1], axis=0))
                    xin = gpool.tile([128, D], BF16, tag="e16")
                    nc.vector.tensor_copy(xin[:], e32[:])
                else:
                    fsrc, fidx, bsrc, bidx = dsrc
                    xin = gpool.tile([128, D], BF16, tag="e16")
                    for src_t, sidx, lo in ((fsrc, fidx, 0), (bsrc, bidx, H)):
                        if sidx is None:
                            nc.gpsimd.dma_start(xin[:, lo:lo + H],
                                                src_t[128 * c:128 * (c + 1), :])
                        else:
                            nc.gpsimd.indirect_dma_start(
                                out=xin[:, lo:lo + H], out_offset=None, in_=src_t[:],
                                in_offset=IndirectOffsetOnAxis(ap=sidx[:, c:c + 1], axis=0))
                xT = tpool.tile([128, KD, 128], BF16, tag="xT")
                for k in range(KD):
                    nc.sync.dma_start_transpose(
                        xT[:, k, :], xin[:, 128 * k:128 * (k + 1)])
                gout = opool.tile([128, G], BF16, tag="gout")
                for n in range(G // 512):
                    ps = ppool.tile([128, 512], F32, name="pps")
                    for k in range(KD):
                        nc.tensor.matmul(
                            ps[:], xT[:, k, :], wsb[s][:, k, 512 * n:512 * (n + 1)],
                            start=(k == 0), stop=(k == KD - 1))
                    nc.vector.tensor_tensor(
                        out=gout[:, 512 * n:512 * (n + 1)], in0=ps[:],
                        in1=bias_sb[s][:, 512 * n:512 * (n + 1)],
                        op=ALU.add)
                nc.gpsimd.dma_start(gx[s][128 * c:128 * (c + 1), :], gout[:])


def _scan_phase(nc, tc, Tn, Bl, TC, RC, KH, scans, whh, gx, hout,
                mask_sb, id_sb):
    gxv = {s: gx[s].ap().rearrange("(b t) d -> b t d", b=Bl) for s in scans}
    houtv = {s: hout[s].ap().rearrange("(b t) d -> b t d", b=Bl) for s in scans}
    NS = G // 512
    with tc.tile_pool(name="sw", bufs=1) as wpool, \
         tc.tile_pool(name="sgx", bufs=6) as gxpool, \
         tc.tile_pool(name="sst", bufs=1) as stpool, \
         tc.tile_pool(name="sps", bufs=2, space="PSUM") as pspool, \
         tc.tile_pool(name="swk", bufs=3) as wkpool, \
         tc.tile_pool(name="shT", bufs=3) as htpool, \
         tc.tile_pool(name="srng", bufs=3) as rpool:
        wsb, c_st, hT = {}, {}, {}
        for s in scans:
            wsb[s] = wpool.tile([128, KH, G], BF16, tag=f"whh{s}", name=f"whh_{s}")
            for k in range(KH):
                nc.gpsimd.dma_start(wsb[s][:, k, :], whh[s][128 * k:128 * (k + 1), :])
            c_st[s] = stpool.tile([Bl, H], F32, tag=f"c{s}", name=f"c_{s}")
            nc.vector.memset(c_st[s][:], 0.0)
            hT[s] = htpool.tile([128, KH * Bl], BF16, tag="hT", name="hT0")
            nc.vector.memset(hT[s][:], 0.0)
        gxc = {s: None for s in scans}
        ring = {s: None for s in scans}
        gsum = {}
        for t in range(Tn):
            # phase 1: recurrent matmuls; per-slice gx adds on VectorE drain
            # PSUM while later matmul groups still run
            for s in scans:
                if t % TC == 0:
                    gxc[s] = gxpool.tile([Bl, TC, G], BF16, tag="gx", name="gxc")
                    if "gxdma" not in ABLATE:
                        nc.gpsimd.dma_start(gxc[s][:], gxv[s][:, t:t + TC, :])
                g_ps = pspool.tile([Bl, G], F32, tag="ps", name="g_ps")
                gsum[s] = wkpool.tile([Bl, G], F32, tag="gsum", name="gsum")
                for n in (0, 1, 3, 2) if "mm" not in ABLATE else ():
                    for k in range(KH):
                        nc.tensor.matmul(g_ps[:, 512 * n:512 * (n + 1)],
                                         hT[s][:, Bl * k:Bl * (k + 1)],
                                         wsb[s][:, k, 512 * n:512 * (n + 1)],
                                         start=(k == 0), stop=(k == KH - 1))
                    nc.vector.tensor_tensor(
                        out=gsum[s][:, 512 * n:512 * (n + 1)],
                        in0=g_ps[:, 512 * n:512 * (n + 1)],
                        in1=gxc[s][:, t % TC, 512 * n:512 * (n + 1)], op=ALU.add)
            # phase 2: op-major across the two scans so neither engine's
            # in-order stream serializes one scan behind the other
            gact, t1, t2, tch, h16 = {}, {}, {}, {}, {}
            for s in scans:
                if t % RC == 0:
                    ring[s] = rpool.tile([Bl, RC, H], BF16, tag="ring", name="ring")
                gact[s] = wkpool.tile([Bl, G], F32, tag="gact", name="gact")
            for s in scans:
                nc.scalar.activation(gact[s][:, 0:2 * H], gsum[s][:, 0:2 * H], AF.Sigmoid)
            for s in scans:
                nc.scalar.activation(gact[s][:, 3 * H:G], gsum[s][:, 3 * H:G], AF.Tanh)
            for s in scans:
                nc.scalar.activation(gact[s][:, 2 * H:3 * H], gsum[s][:, 2 * H:3 * H], AF.Sigmoid)
            for s in scans:
                t1[s] = wkpool.tile([Bl, H], F32, tag="t1", name="t1")
                nc.vector.tensor_tensor(out=t1[s][:], in0=gact[s][:, H:2 * H],
                                        in1=c_st[s][:], op=ALU.mult)
            for s in scans:
                t2[s] = wkpool.tile([Bl, H], F32, tag="t2", name="t2")
                nc.vector.tensor_tensor(out=t2[s][:], in0=gact[s][:, 0:H],
                                        in1=gact[s][:, 3 * H:G], op=ALU.mult)
            for s in scans:
                nc.vector.tensor_tensor(out=c_st[s][:], in0=t1[s][:], in1=t2[s][:],
                                        op=ALU.add)
            for s in scans:
                tch[s] = wkpool.tile([Bl, H], F32, tag="tch", name="tch")
                nc.scalar.activation(tch[s][:], c_st[s][:], AF.Tanh)
            for s in scans:
                h16[s] = wkpool.tile([Bl, H], BF16, tag="h16", name="h16")
                nc.vector.tensor_tensor(out=h16[s][:], in0=gact[s][:, 2 * H:3 * H],
                                        in1=tch[s][:], op=ALU.mult)
            for s in scans:
                hT_ps = pspool.tile([128, KH * Bl], F32, tag="ps", name="hT_ps")
                for k in range(KH if "trans" not in ABLATE else 0):
                    nc.tensor.matmul(hT_ps[:, Bl * k:Bl * (k + 1)],
                                     h16[s][:, 128 * k:128 * (k + 1)], id_sb[:],
                                     start=True, stop=True)
                hTn = htpool.tile([128, KH * Bl], BF16, tag="hT", name="hTn")
                nc.scalar.activation(hTn[:], hT_ps[:], AF.Copy)
                hT[s] = hTn
            for s in scans:
                nc.vector.tensor_scalar_mul(ring[s][:, t % RC, :], h16[s][:],
                                            mask_sb[:, t:t + 1])
                if (t + 1) % RC == 0:
                    t0r = t + 1 - RC
                    if "ring" not in ABLATE:
                        nc.gpsimd.dma_start(houtv[s][:, t0r:t0r + RC, :], ring[s][:])


def _prep_inputs(inputs, Tn=T, Bl=BL, ncores=NC):
    """Host-side sharding + weight preprocessing. Returns per-core in_maps."""
    x = np.asarray(inputs["x"]).astype(np.int32)
    lengths = np.asarray(inputs["lengths"]).astype(np.int32)
    emb = np.asarray(inputs["emb"], dtype=np.float32)
    ntok = Bl * Tn

    com = {"emb": emb, "ident": np.eye(16, dtype=ml_dtypes.bfloat16)}
    for s in ("f1", "b1", "f2", "b2"):
        w_ih = np.asarray(inputs[f"W_ih_{s}"], np.float32)[_GATE_PERM]
        w_hh = np.asarray(inputs[f"W_hh_{s}"], np.float32)[_GATE_PERM]
        b = np.asarray(inputs[f"b_{s}"], np.float32)[_GATE_PERM]
        com[f"wihT_{s}"] = np.ascontiguousarray(w_ih.T).astype(ml_dtypes.bfloat16)
        com[f"whhT_{s}"] = np.ascontiguousarray(w_hh.T).astype(ml_dtypes.bfloat16)
        com[f"bias_{s}"] = np.tile(b.reshape(1, G), (128, 1))
    com["wclsT"] = np.ascontiguousarray(
        np.asarray(inputs["W_cls"], np.float32).T).astype(ml_dtypes.bfloat16)
    com["bcls"] = np.asarray(inputs["b_cls"], np.float32).reshape(TAGS, 1)

    def chunked(a):  # [ntok] -> [128, ntok//128] with chunk c in column c
        return np.ascontiguousarray(a.reshape(-1).reshape(ntok // 128, 128).T)

    in_maps = []
    for c in range(ncores):
        xs = x[Bl * c:Bl * (c + 1), :Tn]
        ls = np.minimum(lengths[Bl * c:Bl * (c + 1)], Tn)
        ts = np.arange(Tn)[None, :]
        rev = np.where(ts < ls[:, None], ls[:, None] - 1 - ts, ts)  # [Bl,Tn]
        xrev = np.take_along_axis(xs, rev, axis=1)
        flat_rev = (np.arange(Bl)[:, None] * Tn + rev).astype(np.int32)
        m = {
            "xf_idx": chunked(xs),
            "xb_idx": chunked(xrev),
            "rev128": chunked(flat_rev),
            "rev16": np.ascontiguousarray(flat_rev),
            "mask": (ts < ls[:, None]).astype(np.float32),
        }
        m.update(com)
        in_maps.append(m)
    return in_maps


_CACHED = {}


def kernel(**inputs) -> np.ndarray:
    if "nc" not in _CACHED:
        nc = bacc.Bacc("TRN2", target_bir_lowering=False, debug=False,
                       num_devices=NC)
        _build(nc)
        nc.compile()
        _CACHED["nc"] = nc
    nc = _CACHED["nc"]
    in_maps = _prep_inputs(inputs)
    res = run_bass_kernel_spmd(nc, in_maps, core_ids=list(range(NC)), trace=False)
    outs = []
    for c in range(NC):
        lt = res.results[c]["logitsT"]  # [TAGS, ntok]
        outs.append(np.ascontiguousarray(lt.T.reshape(BL, T, TAGS)))
    return np.concatenate(outs, axis=0).astype(np.float32)
